# revision 13
# baseline (speedup 1.0000x reference)
"""Bahdanau attention with per-example gathered windows, on 8 trn2 NeuronCores.

Computation per example b (B=64, T=4096, U=512, L=128):
  window  = values[b, max(pos-64,0) : min(pos+64,T)]  zero-padded to L rows
  h       = tanh(window @ W1 + b1 + query[b] @ W2 + b2)        [L, U]
  score   = h @ V (+ bV -- dropped: softmax is shift-invariant) [L, 1]
  attn    = softmax(score, axis=0)                              [L, 1]
  context = sum_l attn[l] * window[l]                           [U]

Sharding: data-parallel over batch, 8 examples per core. The gather is
data-driven (indirect DMA with host-computed row indices + a zero row
appended to the values shard), so one SPMD program serves all cores.

Layout strategy per core (BP=8 examples, C=4 chunks of 128 over U):
  va_all  [128(l), 8(ex), 512(u)]   gathered windows (indirect DMA)
  vaT     [128(u'), c, ex, 128(l)]  PE-transposed windows
  hT      [128(v'), c, (ex,l)]      = tanh(sum_u W1[u,v] vaT[u,...] + bias)
     bias folded into the matmul accumulation with a block-diagonal
     ones matrix E4: psum += qW2b[b,v] * E4[b,(e,l)]
  scores  [1, (ex,l)]  = sum_v V[v] hT[v,...]
  softmax on the [1, 512] row per group of 4 examples (segmented sums)
  context [ex, 512] = attn-column (PE-transposed) as lhsT against va_all

All heavy matmuls run as float32r (single-pass FP22 truncation) with
moving free dim 512 to hit the 1 cycle/row PE rate.
"""

import numpy as np

B, T, U, L = 64, 4096, 512, 128
NCORES = 8
BP = B // NCORES          # examples per core
C = U // 128              # 128-wide chunks over U
G = 2                     # example groups per core
GE = BP // G              # examples per group (4)
HALF = L // 2

_PROGRAM = None           # cached (nc, meta)


def _build_program(stage="full"):
    import concourse.bass as bass
    import concourse.mybir as mybir
    import concourse.tile as tile
    from concourse import bacc

    f32 = mybir.dt.float32
    f32r = mybir.dt.float32r
    i32 = mybir.dt.int32
    AF = mybir.ActivationFunctionType
    AX = mybir.AxisListType

    nc = bacc.Bacc("TRN2", target_bir_lowering=False)

    valuesz = nc.declare_dram_parameter("valuesz", [BP * T + 1, U], f32r, isOutput=False)
    query = nc.declare_dram_parameter("query", [BP, U], f32r, isOutput=False)
    W1 = nc.declare_dram_parameter("W1", [U, U], f32r, isOutput=False)
    W2 = nc.declare_dram_parameter("W2", [U, U], f32r, isOutput=False)
    Vr = nc.declare_dram_parameter("Vr", [128, C], f32r, isOutput=False)
    b12 = nc.declare_dram_parameter("b12", [1, U], f32r, isOutput=False)
    E4 = nc.declare_dram_parameter("E4", [GE, GE * L], f32r, isOutput=False)
    ident = nc.declare_dram_parameter("ident", [128, 128], f32r, isOutput=False)
    ones1 = nc.declare_dram_parameter("ones1", [1, GE], f32r, isOutput=False)
    Zz = nc.declare_dram_parameter("Zz", [128, BP * GE], f32r, isOutput=False)
    gidx = nc.declare_dram_parameter("gidx", [L, BP], i32, isOutput=False)
    STAGES = ["vaT", "qw2b", "h", "scores", "softmax", "attnT", "ctx", "full"]
    lvl = STAGES.index(stage)
    if stage == "full":
        ctx_out = nc.declare_dram_parameter("ctx", [BP, U], f32, isOutput=True)
        attn_out = nc.declare_dram_parameter("attn", [BP, L], f32, isOutput=True)
    elif stage == "vaT":
        dbg = nc.declare_dram_parameter("dbg", [128, C * BP * L], f32r, isOutput=True)
    elif stage == "qw2b":
        dbg = nc.declare_dram_parameter("dbg", [G, GE, U], f32r, isOutput=True)
    elif stage == "h":
        dbg = nc.declare_dram_parameter("dbg", [G, 128, C * GE * L], f32r, isOutput=True)
    elif stage == "scores":
        dbg = nc.declare_dram_parameter("dbg", [G, 1, GE * L], f32, isOutput=True)
    elif stage == "softmax":
        dbg = nc.declare_dram_parameter("dbg", [G, 1, GE * L], f32, isOutput=True)
    elif stage == "attnT":
        dbg = nc.declare_dram_parameter("dbg", [G, 128, GE], f32, isOutput=True)
    elif stage == "ctx":
        dbg = nc.declare_dram_parameter("dbg", [G, GE, U], f32, isOutput=True)

    with tile.TileContext(nc) as tc:
        with (
            tc.tile_pool(name="const", bufs=1) as const,
            tc.tile_pool(name="work", bufs=1) as work,
            tc.tile_pool(name="hT", bufs=2) as hT_pool,
            tc.tile_pool(name="small", bufs=2) as small,
            tc.tile_pool(name="ptr", bufs=2, space="PSUM") as ptr_pool,
            tc.tile_pool(name="ph", bufs=2, space="PSUM") as ph_pool,
            tc.tile_pool(name="pmix", bufs=2, space="PSUM") as pmix_pool,
            tc.tile_pool(name="pat", bufs=1, space="PSUM") as pat_pool,
            tc.tile_pool(name="pctx", bufs=1, space="PSUM") as pctx_pool,
        ):
            # ---- constants -------------------------------------------------
            gidx_sb = const.tile([L, BP], i32)
            nc.sync.dma_start(out=gidx_sb[:], in_=gidx[:])
            q_sb = const.tile([BP, U], f32r)
            nc.sync.dma_start(out=q_sb[:], in_=query[:])
            W2_sb = const.tile([128, C, U], f32r)
            nc.sync.dma_start(out=W2_sb[:], in_=W2[:, :].rearrange("(c p) v -> p c v", p=128))
            W1_sb = const.tile([128, C, U], f32r)
            nc.sync.dma_start(out=W1_sb[:], in_=W1[:, :].rearrange("(c p) v -> p c v", p=128))
            Vr_sb = const.tile([128, C], f32r)
            nc.sync.dma_start(out=Vr_sb[:], in_=Vr[:])
            b12_sb = const.tile([1, U], f32r)
            nc.sync.dma_start(out=b12_sb[:], in_=b12[:])
            E4_sb = const.tile([GE, GE * L], f32r)
            nc.sync.dma_start(out=E4_sb[:], in_=E4[:])
            id_sb = const.tile([128, 128], f32r)
            nc.sync.dma_start(out=id_sb[:], in_=ident[:])
            ones_sb = const.tile([1, GE], f32r)
            nc.sync.dma_start(out=ones_sb[:], in_=ones1[:])
            id1_sb = const.tile([1, 1], f32)
            nc.gpsimd.memset(id1_sb[:], 1.0)

            # ---- gather windows (data-driven, zero row pads) ---------------
            va_all = work.tile([128, BP, U], f32r)
            for b in range(BP):
                nc.gpsimd.indirect_dma_start(
                    out=va_all[:, b, :],
                    out_offset=None,
                    in_=valuesz[:, :],
                    in_offset=bass.IndirectOffsetOnAxis(ap=gidx_sb[:, b : b + 1], axis=0),
                )

            # ---- qT -> qW2b per group --------------------------------------
            pqT = pmix_pool.tile([128, C * BP], f32r, tag="pmix")
            for c in range(C):
                nc.tensor.transpose(
                    out=pqT[:, c * BP : (c + 1) * BP],
                    in_=q_sb[:, c * 128 : (c + 1) * 128],
                    identity=id_sb[0:BP, 0:BP],
                )
            qT_sb = const.tile([128, C * BP], f32r)
            nc.vector.tensor_copy(out=qT_sb[:], in_=pqT[:])

            qW2b_sb = []
            for g in range(G if lvl >= 1 else 0):
                pq = pmix_pool.tile([GE, U], f32, tag="pmix")
                for c in range(C):
                    nc.tensor.matmul(
                        out=pq[:],
                        lhsT=qT_sb[:, c * BP + g * GE : c * BP + (g + 1) * GE],
                        rhs=W2_sb[:, c, :],
                        start=(c == 0),
                        stop=False,
                    )
                nc.tensor.matmul(
                    out=pq[:],
                    lhsT=ones_sb[:],
                    rhs=b12_sb[:],
                    start=False,
                    stop=True,
                )
                qg = const.tile([GE, U], f32r, tag=f"qW2b{g}")
                nc.vector.tensor_copy(out=qg[:], in_=pq[:])
                qW2b_sb.append(qg)
                if stage == "qw2b":
                    nc.sync.dma_start(out=dbg[g], in_=qg[:])

            # ---- transpose windows -----------------------------------------
            vaT = work.tile([128, C, BP, L], f32r)
            for b in range(BP):
                ptr = ptr_pool.tile([128, U], f32r, tag="ptr")
                for j in range(C):
                    nc.tensor.transpose(
                        out=ptr[:, j * 128 : (j + 1) * 128],
                        in_=va_all[:, b, j * 128 : (j + 1) * 128],
                        identity=id_sb[:, :],
                    )
                nc.vector.tensor_copy(
                    out=vaT[:, :, b, :],
                    in_=ptr[:, :].rearrange("p (j l) -> p j l", j=C),
                )
            if stage == "vaT":
                nc.sync.dma_start(out=dbg[:], in_=vaT[:].rearrange("p c b l -> p (c b l)"))

            # ---- per group: h, scores, softmax, context --------------------
            # Z[:, b, :] is a [128(l), GE] matrix whose only nonzero column
            # (column e = b % GE) holds example b's attention weights — the
            # block-diagonal lhsT that lets one [GE, U] psum accumulate all
            # GE per-example weighted sums at base partition 0.
            Z = const.tile([128, BP, GE], f32r)
            if lvl >= 5:
                nc.sync.dma_start(out=Z[:], in_=Zz[:].rearrange("p (b e) -> p b e", e=GE))
            for g in range(G if lvl >= 2 else 0):
                hT_g = hT_pool.tile([128, C, GE * L], f32r, tag="hT")
                for c in range(C):
                    ph = ph_pool.tile([128, GE * L], f32, tag="ph")
                    for j in range(C):
                        nc.tensor.matmul(
                            out=ph[:],
                            lhsT=W1_sb[:, j, c * 128 : (c + 1) * 128],
                            rhs=vaT[:, j, g * GE : (g + 1) * GE, :],
                            start=(j == 0),
                            stop=False,
                        )
                    nc.tensor.matmul(
                        out=ph[:],
                        lhsT=qW2b_sb[g][:, c * 128 : (c + 1) * 128],
                        rhs=E4_sb[:],
                        start=False,
                        stop=True,
                    )
                    nc.scalar.activation(out=hT_g[:, c, :], in_=ph[:], func=AF.Tanh)
                if stage == "h":
                    nc.sync.dma_start(out=dbg[g], in_=hT_g[:].rearrange("p c n -> p (c n)"))
                    continue

                ps = pmix_pool.tile([1, GE * L], f32, tag="pmix")
                for c in range(C):
                    nc.tensor.matmul(
                        out=ps[:],
                        lhsT=Vr_sb[:, c : c + 1],
                        rhs=hT_g[:, c, :],
                        start=(c == 0),
                        stop=(c == C - 1),
                    )

                if stage == "scores":
                    sc_sb = small.tile([1, GE * L], f32, tag="sc")
                    nc.vector.tensor_copy(out=sc_sb[:], in_=ps[:])
                    nc.sync.dma_start(out=dbg[g], in_=sc_sb[:])
                    continue
                e_g = small.tile([1, GE * L], f32, tag="e")
                nc.scalar.activation(out=e_g[:], in_=ps[:], func=AF.Exp)
                s_g = small.tile([1, GE], f32, tag="s")
                nc.vector.reduce_sum(
                    out=s_g[:],
                    in_=e_g[0:1, :].rearrange("p (e l) -> p e l", l=L),
                    axis=AX.X,
                )
                r_g = small.tile([1, GE], f32, tag="r")
                nc.vector.reciprocal(r_g[:], s_g[:])
                attn_g = small.tile([1, GE * L], f32, tag="attn")
                for e in range(GE):
                    nc.vector.tensor_scalar_mul(
                        out=attn_g[0:1, e * L : (e + 1) * L],
                        in0=e_g[0:1, e * L : (e + 1) * L],
                        scalar1=r_g[0:1, e : e + 1],
                    )
                if stage == "softmax":
                    nc.sync.dma_start(out=dbg[g], in_=attn_g[:])
                    continue
                pat = pat_pool.tile([128, GE], f32, tag="pat")
                for e in range(GE):
                    b = g * GE + e
                    if stage == "full":
                        nc.sync.dma_start(
                            out=attn_out[b : b + 1, :],
                            in_=attn_g[0:1, e * L : (e + 1) * L],
                        )
                    nc.tensor.transpose(
                        out=pat[:, e : e + 1],
                        in_=attn_g[0:1, e * L : (e + 1) * L],
                        identity=id1_sb[:],
                    )
                for e in range(GE):
                    b = g * GE + e
                    nc.vector.tensor_copy(out=Z[:, b, e : e + 1], in_=pat[:, e : e + 1])
                if stage == "attnT":
                    zc = small.tile([128, GE], f32, tag="zc")
                    nc.vector.tensor_copy(out=zc[:], in_=pat[:, :])
                    nc.sync.dma_start(out=dbg[g], in_=zc[:])
                    continue
                pctx = pctx_pool.tile([GE, U], f32, tag="pctx")
                for e in range(GE):
                    b = g * GE + e
                    nc.tensor.matmul(
                        out=pctx[:],
                        lhsT=Z[:, b, :],
                        rhs=va_all[:, b, :],
                        start=(e == 0),
                        stop=(e == GE - 1),
                    )
                ctx_sb = small.tile([GE, U], f32, tag="ctx")
                nc.vector.tensor_copy(out=ctx_sb[:], in_=pctx[:])
                if stage == "ctx":
                    nc.sync.dma_start(out=dbg[g], in_=ctx_sb[:])
                else:
                    nc.sync.dma_start(out=ctx_out[g * GE : (g + 1) * GE, :], in_=ctx_sb[:])

    nc.compile()
    return nc


def _get_program():
    global _PROGRAM
    if _PROGRAM is None:
        _PROGRAM = _build_program()
    return _PROGRAM


def make_in_maps(query, values, pos, W1, b1, W2, b2, V, bV):
    """Host-side shard + index prep. Pure numpy, no device work."""
    query = np.ascontiguousarray(np.asarray(query, np.float32))
    values = np.asarray(values, np.float32)
    pos = np.asarray(pos, np.int32)
    W1 = np.ascontiguousarray(np.asarray(W1, np.float32))
    W2 = np.ascontiguousarray(np.asarray(W2, np.float32))
    Vr = np.ascontiguousarray(np.asarray(V, np.float32).reshape(C, 128).T)
    b12 = np.ascontiguousarray(
        (np.asarray(b1, np.float32) + np.asarray(b2, np.float32)).reshape(1, U)
    )
    E4 = np.zeros((GE, GE * L), np.float32)
    for e in range(GE):
        E4[e, e * L : (e + 1) * L] = 1.0

    ident = np.eye(128, dtype=np.float32)
    ones1 = np.ones((1, GE), np.float32)
    Zz = np.zeros((128, BP * GE), np.float32)
    offs = np.arange(L, dtype=np.int64)
    in_maps = []
    for core in range(NCORES):
        s = slice(core * BP, (core + 1) * BP)
        vshard = values[s].reshape(BP * T, U)
        valuesz = np.vstack([vshard, np.zeros((1, U), np.float32)])
        p = pos[s].astype(np.int64)
        start = np.maximum(p - HALF, 0)
        end = np.minimum(p + HALF, T)
        xlen = end - start
        # gidx[l, b]: row into valuesz; padded slots hit the zero row
        idx = start[None, :] + offs[:, None] + np.arange(BP)[None, :] * T
        idx = np.where(offs[:, None] < xlen[None, :], idx, BP * T)
        in_maps.append(
            {
                "valuesz": np.ascontiguousarray(valuesz),
                "query": np.ascontiguousarray(query[s]),
                "W1": W1,
                "W2": W2,
                "Vr": Vr,
                "b12": b12,
                "E4": E4,
                "ident": ident,
                "ones1": ones1,
                "Zz": Zz,
                "gidx": np.ascontiguousarray(idx.astype(np.int32)),
            }
        )
    return in_maps


def run_on_hw(in_maps, trace=False, **kw):
    from concourse.bass_utils import run_bass_kernel_spmd

    nc = _get_program()
    return run_bass_kernel_spmd(nc, in_maps, list(range(NCORES)), trace=trace, **kw)


def kernel(query, values, pos, W1, b1, W2, b2, V, bV):
    in_maps = make_in_maps(query, values, pos, W1, b1, W2, b2, V, bV)
    res = run_on_hw(in_maps).results
    ctx = np.concatenate([r["ctx"] for r in res], axis=0)
    attn = np.concatenate([r["attn"] for r in res], axis=0)
    return ctx, attn.reshape(B, L, 1)


# revision 15
# speedup vs baseline: 1.0937x; 1.0937x over previous
"""Bahdanau attention with per-example gathered windows, on 8 trn2 NeuronCores.

Computation per example b (B=64, T=4096, U=512, L=128):
  window  = values[b, max(pos-64,0) : min(pos+64,T)]  zero-padded to L rows
  h       = tanh(window @ W1 + b1 + query[b] @ W2 + b2)        [L, U]
  score   = h @ V (+ bV -- dropped: softmax is shift-invariant) [L, 1]
  attn    = softmax(score, axis=0)                              [L, 1]
  context = sum_l attn[l] * window[l]                           [U]

Sharding: data-parallel over batch, 8 examples per core. The gather is
data-driven (indirect DMA with host-computed row indices + a zero row
appended to the values shard), so one SPMD program serves all cores.

Layout strategy per core (BP=8 examples, C=4 chunks of 128 over U):
  va[b]    [128(l), 512(u)]          gathered window (indirect DMA)
  vaT[g]   [128(u'), c, e, 128(l)]   PE-transposed windows, group of 4
  hT       [128(v'), c, (e,l)]       = tanh(sum_u W1[u,v] vaT[u,...] + bias)
     bias folded into the matmul accumulation with a block-diagonal
     ones matrix E4: psum += qW2b[b,v] * E4[b,(e,l)]
  scores   [1, (e,l)]  = sum_v V[v] hT[v,...]
  softmax on the [1, 512] row per group of 4 examples (segmented sums)
  context  [e, 512] accumulated from per-example masked-column lhsT
     Z[b] ([128, GE], only column b%GE nonzero = attn weights) against va[b]

All heavy matmuls run as float32r (single-pass FP22 truncation) with
moving free dim 512 to hit the 1 cycle/row PE rate. Tiles are split
per-example / per-group so Tile's per-tile dependency tracking doesn't
serialize consumers behind unrelated producers.
"""

import numpy as np

B, T, U, L = 64, 4096, 512, 128
NCORES = 8
BP = B // NCORES          # examples per core
C = U // 128              # 128-wide chunks over U
G = 2                     # example groups per core
GE = BP // G              # examples per group (4)
HALF = L // 2

_PROGRAM = None


def _build_program():
    import concourse.bass as bass
    import concourse.mybir as mybir
    import concourse.tile as tile
    from concourse import bacc

    f32 = mybir.dt.float32
    f32r = mybir.dt.float32r
    i32 = mybir.dt.int32
    AF = mybir.ActivationFunctionType
    AX = mybir.AxisListType

    nc = bacc.Bacc("TRN2", target_bir_lowering=False)

    valuesz = nc.declare_dram_parameter("valuesz", [BP * T + 1, U], f32r, isOutput=False)
    query = nc.declare_dram_parameter("query", [BP, U], f32r, isOutput=False)
    W1 = nc.declare_dram_parameter("W1", [U, U], f32r, isOutput=False)
    W2 = nc.declare_dram_parameter("W2", [U, U], f32r, isOutput=False)
    Vr = nc.declare_dram_parameter("Vr", [128, C], f32r, isOutput=False)
    b12 = nc.declare_dram_parameter("b12", [1, U], f32r, isOutput=False)
    E4 = nc.declare_dram_parameter("E4", [GE, GE * L], f32r, isOutput=False)
    ident = nc.declare_dram_parameter("ident", [128, 128], f32r, isOutput=False)
    ones1 = nc.declare_dram_parameter("ones1", [1, GE], f32r, isOutput=False)
    Zz = nc.declare_dram_parameter("Zz", [128, BP * GE], f32r, isOutput=False)
    gidx = nc.declare_dram_parameter("gidx", [L, BP], i32, isOutput=False)
    ctx_out = nc.declare_dram_parameter("ctx", [BP, U], f32, isOutput=True)
    attn_out = nc.declare_dram_parameter("attn", [BP, L], f32, isOutput=True)

    with tile.TileContext(nc) as tc:
        with (
            tc.tile_pool(name="const", bufs=1) as const,
            tc.tile_pool(name="work", bufs=1) as work,
            tc.tile_pool(name="hT", bufs=2) as hT_pool,
            tc.tile_pool(name="small", bufs=2) as small,
            tc.tile_pool(name="ptr", bufs=2, space="PSUM") as ptr_pool,
            tc.tile_pool(name="ph", bufs=2, space="PSUM") as ph_pool,
            tc.tile_pool(name="pmix", bufs=2, space="PSUM") as pmix_pool,
            tc.tile_pool(name="ptail", bufs=2, space="PSUM") as ptail_pool,
        ):
            # ---- gather first: gidx, then the 8 indirect window gathers ----
            gidx_sb = const.tile([L, BP], i32)
            nc.sync.dma_start(out=gidx_sb[:], in_=gidx[:])
            va = []
            for b in range(BP):
                vb = work.tile([128, U], f32r, tag=f"va{b}", name=f"va{b}")
                nc.gpsimd.indirect_dma_start(
                    out=vb[:],
                    out_offset=None,
                    in_=valuesz[:, :],
                    in_offset=bass.IndirectOffsetOnAxis(ap=gidx_sb[:, b : b + 1], axis=0),
                )
                va.append(vb)

            # ---- constants (DMAs spread over the two HWDGE engines) --------
            q_sb = const.tile([BP, U], f32r)
            nc.sync.dma_start(out=q_sb[:], in_=query[:])
            id_sb = const.tile([128, 128], f32r)
            nc.sync.dma_start(out=id_sb[:], in_=ident[:])
            W2_sb = const.tile([128, C, U], f32r)
            nc.scalar.dma_start(out=W2_sb[:], in_=W2[:, :].rearrange("(c p) v -> p c v", p=128))
            W1_sb = const.tile([128, C, U], f32r)
            nc.scalar.dma_start(out=W1_sb[:], in_=W1[:, :].rearrange("(c p) v -> p c v", p=128))
            Vr_sb = const.tile([128, C], f32r)
            nc.sync.dma_start(out=Vr_sb[:], in_=Vr[:])
            b12_sb = const.tile([1, U], f32r)
            nc.sync.dma_start(out=b12_sb[:], in_=b12[:])
            E4_sb = const.tile([GE, GE * L], f32r)
            nc.sync.dma_start(out=E4_sb[:], in_=E4[:])
            ones_sb = const.tile([1, GE], f32r)
            nc.sync.dma_start(out=ones_sb[:], in_=ones1[:])
            id1_sb = const.tile([1, 1], f32)
            nc.gpsimd.memset(id1_sb[:], 1.0)
            # per-example masked-attn scaffolds (zero background)
            Zt = []
            for b in range(BP):
                zb = const.tile([128, GE], f32r, tag=f"Z{b}", name=f"Z{b}")
                nc.sync.dma_start(out=zb[:], in_=Zz[:, b * GE : (b + 1) * GE])
                Zt.append(zb)

            # ---- qT -> qW2b per group --------------------------------------
            pqT = pmix_pool.tile([128, C * BP], f32r, tag="pmix")
            for c in range(C):
                nc.tensor.transpose(
                    out=pqT[:, c * BP : (c + 1) * BP],
                    in_=q_sb[:, c * 128 : (c + 1) * 128],
                    identity=id_sb[0:BP, 0:BP],
                )
            qT_sb = const.tile([128, C * BP], f32r)
            nc.vector.tensor_copy(out=qT_sb[:], in_=pqT[:])

            qW2b_sb = []
            for g in range(G):
                pq = pmix_pool.tile([GE, U], f32, tag="pmix")
                for c in range(C):
                    nc.tensor.matmul(
                        out=pq[:],
                        lhsT=qT_sb[:, c * BP + g * GE : c * BP + (g + 1) * GE],
                        rhs=W2_sb[:, c, :],
                        start=(c == 0),
                        stop=False,
                    )
                nc.tensor.matmul(
                    out=pq[:], lhsT=ones_sb[:], rhs=b12_sb[:], start=False, stop=True
                )
                qg = const.tile([GE, U], f32r, tag=f"qW2b{g}", name=f"qW2b{g}")
                nc.vector.tensor_copy(out=qg[:], in_=pq[:])
                qW2b_sb.append(qg)

            # ---- transpose windows (per group destination) -----------------
            vaT = [
                work.tile([128, C, GE, L], f32r, tag=f"vaT{g}", name=f"vaT{g}")
                for g in range(G)
            ]
            for b in range(BP):
                g, e = divmod(b, GE)
                ptr = ptr_pool.tile([128, U], f32r, tag="ptr")
                for j in range(C):
                    nc.tensor.transpose(
                        out=ptr[:, j * 128 : (j + 1) * 128],
                        in_=va[b][:, j * 128 : (j + 1) * 128],
                        identity=id_sb[:, :],
                    )
                nc.vector.tensor_copy(
                    out=vaT[g][:, :, e, :],
                    in_=ptr[:, :].rearrange("p (j l) -> p j l", j=C),
                )

            # ---- per group: h, scores, softmax, context --------------------
            for g in range(G):
                hT_g = hT_pool.tile([128, C, GE * L], f32r, tag="hT")
                for c in range(C):
                    ph = ph_pool.tile([128, GE * L], f32, tag="ph")
                    for j in range(C):
                        nc.tensor.matmul(
                            out=ph[:],
                            lhsT=W1_sb[:, j, c * 128 : (c + 1) * 128],
                            rhs=vaT[g][:, j, :, :],
                            start=(j == 0),
                            stop=False,
                        )
                    nc.tensor.matmul(
                        out=ph[:],
                        lhsT=qW2b_sb[g][:, c * 128 : (c + 1) * 128],
                        rhs=E4_sb[:],
                        start=False,
                        stop=True,
                    )
                    nc.scalar.activation(out=hT_g[:, c, :], in_=ph[:], func=AF.Tanh)

                ps = pmix_pool.tile([1, GE * L], f32, tag="pmix")
                for c in range(C):
                    nc.tensor.matmul(
                        out=ps[:],
                        lhsT=Vr_sb[:, c : c + 1],
                        rhs=hT_g[:, c, :],
                        start=(c == 0),
                        stop=(c == C - 1),
                    )

                e_g = small.tile([1, GE * L], f32, tag="e")
                nc.scalar.activation(out=e_g[:], in_=ps[:], func=AF.Exp)
                s_g = small.tile([1, GE], f32, tag="s")
                nc.vector.reduce_sum(
                    out=s_g[:],
                    in_=e_g[0:1, :].rearrange("p (e l) -> p e l", l=L),
                    axis=AX.X,
                )
                r_g = small.tile([1, GE], f32, tag="r")
                nc.vector.reciprocal(r_g[:], s_g[:])
                attn_g = small.tile([1, GE * L], f32, tag="attn")
                for e in range(GE):
                    nc.vector.tensor_scalar_mul(
                        out=attn_g[0:1, e * L : (e + 1) * L],
                        in0=e_g[0:1, e * L : (e + 1) * L],
                        scalar1=r_g[0:1, e : e + 1],
                    )
                pat = ptail_pool.tile([128, GE], f32, tag="ptail")
                for e in range(GE):
                    b = g * GE + e
                    nc.sync.dma_start(
                        out=attn_out[b : b + 1, :],
                        in_=attn_g[0:1, e * L : (e + 1) * L],
                    )
                    nc.tensor.transpose(
                        out=pat[:, e : e + 1],
                        in_=attn_g[0:1, e * L : (e + 1) * L],
                        identity=id1_sb[:],
                    )
                    nc.vector.tensor_copy(
                        out=Zt[b][:, e : e + 1], in_=pat[:, e : e + 1]
                    )
                pctx = ptail_pool.tile([GE, U], f32, tag="ptail")
                for e in range(GE):
                    b = g * GE + e
                    nc.tensor.matmul(
                        out=pctx[:],
                        lhsT=Zt[b][:, :],
                        rhs=va[b][:, :],
                        start=(e == 0),
                        stop=(e == GE - 1),
                    )
                ctx_sb = small.tile([GE, U], f32, tag="ctx")
                nc.vector.tensor_copy(out=ctx_sb[:], in_=pctx[:])
                nc.sync.dma_start(out=ctx_out[g * GE : (g + 1) * GE, :], in_=ctx_sb[:])

    nc.compile()
    return nc


def _get_program():
    global _PROGRAM
    if _PROGRAM is None:
        _PROGRAM = _build_program()
    return _PROGRAM


def make_in_maps(query, values, pos, W1, b1, W2, b2, V, bV):
    """Host-side shard + index prep. Pure numpy, no device work."""
    query = np.ascontiguousarray(np.asarray(query, np.float32))
    values = np.asarray(values, np.float32)
    pos = np.asarray(pos, np.int32)
    W1 = np.ascontiguousarray(np.asarray(W1, np.float32))
    W2 = np.ascontiguousarray(np.asarray(W2, np.float32))
    Vr = np.ascontiguousarray(np.asarray(V, np.float32).reshape(C, 128).T)
    b12 = np.ascontiguousarray(
        (np.asarray(b1, np.float32) + np.asarray(b2, np.float32)).reshape(1, U)
    )
    E4 = np.zeros((GE, GE * L), np.float32)
    for e in range(GE):
        E4[e, e * L : (e + 1) * L] = 1.0

    ident = np.eye(128, dtype=np.float32)
    ones1 = np.ones((1, GE), np.float32)
    Zz = np.zeros((128, BP * GE), np.float32)
    offs = np.arange(L, dtype=np.int64)
    in_maps = []
    for core in range(NCORES):
        s = slice(core * BP, (core + 1) * BP)
        vshard = values[s].reshape(BP * T, U)
        valuesz = np.vstack([vshard, np.zeros((1, U), np.float32)])
        p = pos[s].astype(np.int64)
        start = np.maximum(p - HALF, 0)
        end = np.minimum(p + HALF, T)
        xlen = end - start
        # gidx[l, b]: row into valuesz; padded slots hit the zero row
        idx = start[None, :] + offs[:, None] + np.arange(BP)[None, :] * T
        idx = np.where(offs[:, None] < xlen[None, :], idx, BP * T)
        in_maps.append(
            {
                "valuesz": np.ascontiguousarray(valuesz),
                "query": np.ascontiguousarray(query[s]),
                "W1": W1,
                "W2": W2,
                "Vr": Vr,
                "b12": b12,
                "E4": E4,
                "ident": ident,
                "ones1": ones1,
                "Zz": Zz,
                "gidx": np.ascontiguousarray(idx.astype(np.int32)),
            }
        )
    return in_maps


def run_on_hw(in_maps, trace=False, **kw):
    from concourse.bass_utils import run_bass_kernel_spmd

    nc = _get_program()
    return run_bass_kernel_spmd(nc, in_maps, list(range(NCORES)), trace=trace, **kw)


def kernel(query, values, pos, W1, b1, W2, b2, V, bV):
    in_maps = make_in_maps(query, values, pos, W1, b1, W2, b2, V, bV)
    res = run_on_hw(in_maps).results
    ctx = np.concatenate([r["ctx"] for r in res], axis=0)
    attn = np.concatenate([r["attn"] for r in res], axis=0)
    return ctx, attn.reshape(B, L, 1)


# revision 20
# speedup vs baseline: 1.1113x; 1.0160x over previous
"""Bahdanau attention with per-example gathered windows, on 8 trn2 NeuronCores.

Computation per example b (B=64, T=4096, U=512, L=128):
  window  = values[b, max(pos-64,0) : min(pos+64,T)]  zero-padded to L rows
  h       = tanh(window @ W1 + b1 + query[b] @ W2 + b2)        [L, U]
  score   = h @ V (+ bV -- dropped: softmax is shift-invariant) [L, 1]
  attn    = softmax(score, axis=0)                              [L, 1]
  context = sum_l attn[l] * window[l]                           [U]

Sharding: data-parallel over batch, 8 examples per core. The gather is
data-driven (indirect DMA with host-computed row indices + a zero row
appended to the values shard), so one SPMD program serves all cores.

Layout strategy per core (BP=8 examples, C=4 chunks of 128 over U):
  va[b]    [128(l), 512(u)]          gathered window (indirect DMA)
  vaT[g]   [128(u'), c, e, 128(l)]   PE-transposed windows, group of 4
  hT       [128(v'), c, (e,l)]       = tanh(sum_u W1[u,v] vaT[u,...] + bias)
     bias folded into the matmul accumulation with a block-diagonal
     ones matrix E4: psum += qW2b[b,v] * E4[b,(e,l)]
  scores   [1, (e,l)]  = sum_v V[v] hT[v,...]
  softmax on the [1, 512] row per group of 4 examples (segmented sums)
  context  [e, 512] accumulated from per-example masked-column lhsT
     Z[b] ([128, GE], only column b%GE nonzero = attn weights) against va[b]

All heavy matmuls run as float32r (single-pass FP22 truncation) with
moving free dim 512 to hit the 1 cycle/row PE rate. Tiles are split
per-example / per-group so Tile's per-tile dependency tracking doesn't
serialize consumers behind unrelated producers.
"""

import numpy as np

B, T, U, L = 64, 4096, 512, 128
NCORES = 8
BP = B // NCORES          # examples per core
C = U // 128              # 128-wide chunks over U
G = 2                     # example groups per core
GE = BP // G              # examples per group (4)
HALF = L // 2

_PROGRAM = None


def _build_program():
    import concourse.bass as bass
    import concourse.mybir as mybir
    import concourse.tile as tile
    from concourse import bacc

    f32 = mybir.dt.float32
    f32r = mybir.dt.float32r
    i32 = mybir.dt.int32
    AF = mybir.ActivationFunctionType
    AX = mybir.AxisListType

    nc = bacc.Bacc("TRN2", target_bir_lowering=False)

    NCONST = 1172
    valuesz = nc.declare_dram_parameter("valuesz", [BP * T + 1, U], f32r, isOutput=False)
    W1 = nc.declare_dram_parameter("W1", [U, U], f32r, isOutput=False)
    W2 = nc.declare_dram_parameter("W2", [U, U], f32r, isOutput=False)
    consts = nc.declare_dram_parameter("consts", [128, NCONST], f32r, isOutput=False)
    gidx = nc.declare_dram_parameter("gidx", [L, BP], i32, isOutput=False)
    Zz = nc.declare_dram_parameter("Zz", [128, BP * GE], f32r, isOutput=False)
    ctx_out = nc.declare_dram_parameter("ctx", [BP, U], f32, isOutput=True)
    attn_out = nc.declare_dram_parameter("attn", [BP, L], f32, isOutput=True)

    with tile.TileContext(nc) as tc:
        with (
            tc.tile_pool(name="const", bufs=1) as const,
            tc.tile_pool(name="work", bufs=1) as work,
            tc.tile_pool(name="hT", bufs=2) as hT_pool,
            tc.tile_pool(name="small", bufs=2) as small,
            tc.tile_pool(name="ptr", bufs=2, space="PSUM") as ptr_pool,
            tc.tile_pool(name="ph", bufs=2, space="PSUM") as ph_pool,
            tc.tile_pool(name="pmix", bufs=2, space="PSUM") as pmix_pool,
            tc.tile_pool(name="ptail", bufs=2, space="PSUM") as ptail_pool,
        ):
            # ---- one packed consts DMA, then the 8 indirect window gathers
            gidx_sb = const.tile([L, BP], i32)
            nc.sync.dma_start(out=gidx_sb[:], in_=gidx[:])
            cs = const.tile([128, NCONST], f32r)
            nc.sync.dma_start(out=cs[:], in_=consts[:])
            id_sb = cs[:, 0:128]
            Vr_sb = cs[:, 128:132]
            E4_sb = cs[0:GE, 140:652]
            q_sb = cs[32 : 32 + BP, 140:652]
            b12_sb = cs[0:1, 656:1168]
            ones_sb = cs[0:1, 1168:1172]
            va = []
            for b in range(BP):
                vb = work.tile([128, U], f32r, tag=f"va{b}", name=f"va{b}")
                nc.gpsimd.indirect_dma_start(
                    out=vb[:],
                    out_offset=None,
                    in_=valuesz[:, :],
                    in_offset=bass.IndirectOffsetOnAxis(ap=gidx_sb[:, b : b + 1], axis=0),
                )
                va.append(vb)

            # ---- weights on the scalar-engine HWDGE queue ------------------
            W1_sb = const.tile([128, C, U], f32r)
            nc.scalar.dma_start(out=W1_sb[:], in_=W1[:, :].rearrange("(c p) v -> p c v", p=128))
            W2_sb = const.tile([128, C, U], f32r)
            nc.scalar.dma_start(out=W2_sb[:], in_=W2[:, :].rearrange("(c p) v -> p c v", p=128))
            id1_sb = const.tile([1, 1], f32)
            nc.gpsimd.memset(id1_sb[:], 1.0)

            # ---- PE warm-up: dummy fp32 matmuls during the DMA lead-in -----
            # HAM unthrottles the PE clock (1.2 -> 2.4 GHz) after ~3.4us of
            # sustained activity; burn that window while inputs stream in.
            warm_sb = const.tile([128, U], f32)
            nc.gpsimd.memset(warm_sb[:], 0.0)
            for w in range(4):
                pw = ph_pool.tile([128, U], f32, tag="ph", name=f"warm{w}")
                nc.tensor.matmul(
                    out=pw[:], lhsT=warm_sb[:, 0:128], rhs=warm_sb[:], start=True, stop=True
                )
            # per-example masked-attn scaffolds (zero background)
            Zt = []
            for b in range(BP):
                zb = const.tile([128, GE], f32r, tag=f"Z{b}", name=f"Z{b}")
                nc.sync.dma_start(out=zb[:], in_=Zz[:, b * GE : (b + 1) * GE])
                Zt.append(zb)

            # ---- qT -> qW2b per group --------------------------------------
            pqT = pmix_pool.tile([128, C * BP], f32r, tag="pmix")
            for c in range(C):
                nc.tensor.transpose(
                    out=pqT[:, c * BP : (c + 1) * BP],
                    in_=q_sb[:, c * 128 : (c + 1) * 128],
                    identity=id_sb[32 : 32 + BP, 32 : 32 + BP],
                )
            qT_sb = const.tile([128, C * BP], f32r)
            nc.vector.tensor_copy(out=qT_sb[:], in_=pqT[:])

            qW2b_sb = []
            for g in range(G):
                pq = pmix_pool.tile([GE, U], f32, tag="pmix")
                for c in range(C):
                    nc.tensor.matmul(
                        out=pq[:],
                        lhsT=qT_sb[:, c * BP + g * GE : c * BP + (g + 1) * GE],
                        rhs=W2_sb[:, c, :],
                        start=(c == 0),
                        stop=False,
                    )
                nc.tensor.matmul(
                    out=pq[:], lhsT=ones_sb, rhs=b12_sb, start=False, stop=True
                )
                qg = const.tile([GE, U], f32r, tag=f"qW2b{g}", name=f"qW2b{g}")
                nc.vector.tensor_copy(out=qg[:], in_=pq[:])
                qW2b_sb.append(qg)

            # ---- transpose windows (per group destination) -----------------
            vaT = [
                work.tile([128, C, GE, L], f32r, tag=f"vaT{g}", name=f"vaT{g}")
                for g in range(G)
            ]
            for b in range(BP):
                g, e = divmod(b, GE)
                ptr = ptr_pool.tile([128, U], f32r, tag="ptr")
                for j in range(C):
                    nc.tensor.transpose(
                        out=ptr[:, j * 128 : (j + 1) * 128],
                        in_=va[b][:, j * 128 : (j + 1) * 128],
                        identity=id_sb,
                    )
                nc.vector.tensor_copy(
                    out=vaT[g][:, :, e, :],
                    in_=ptr[:, :].rearrange("p (j l) -> p j l", j=C),
                )

            # ---- per group: h, scores, softmax, context --------------------
            for g in range(G):
                hT_g = hT_pool.tile([128, C, GE * L], f32r, tag="hT")
                for c in range(C):
                    ph = ph_pool.tile([128, GE * L], f32, tag="ph")
                    for j in range(C):
                        nc.tensor.matmul(
                            out=ph[:],
                            lhsT=W1_sb[:, j, c * 128 : (c + 1) * 128],
                            rhs=vaT[g][:, j, :, :],
                            start=(j == 0),
                            stop=False,
                        )
                    nc.tensor.matmul(
                        out=ph[:],
                        lhsT=qW2b_sb[g][:, c * 128 : (c + 1) * 128],
                        rhs=E4_sb,
                        start=False,
                        stop=True,
                    )
                    nc.scalar.activation(out=hT_g[:, c, :], in_=ph[:], func=AF.Tanh)

                ps = pmix_pool.tile([1, GE * L], f32, tag="pmix")
                for c in range(C):
                    nc.tensor.matmul(
                        out=ps[:],
                        lhsT=Vr_sb[:, c : c + 1],
                        rhs=hT_g[:, c, :],
                        start=(c == 0),
                        stop=(c == C - 1),
                    )

                e_g = small.tile([1, GE * L], f32, tag="e")
                nc.scalar.activation(out=e_g[:], in_=ps[:], func=AF.Exp)
                s_g = small.tile([1, GE], f32, tag="s")
                nc.vector.reduce_sum(
                    out=s_g[:],
                    in_=e_g[0:1, :].rearrange("p (e l) -> p e l", l=L),
                    axis=AX.X,
                )
                r_g = small.tile([1, GE], f32, tag="r")
                nc.vector.reciprocal(r_g[:], s_g[:])
                attn_g = small.tile([1, GE * L], f32, tag="attn")
                for e in range(GE):
                    nc.vector.tensor_scalar_mul(
                        out=attn_g[0:1, e * L : (e + 1) * L],
                        in0=e_g[0:1, e * L : (e + 1) * L],
                        scalar1=r_g[0:1, e : e + 1],
                    )
                nc.sync.dma_start(
                    out=attn_out[g * GE : (g + 1) * GE, :].rearrange("b l -> (b l)"),
                    in_=attn_g[0:1, :],
                )
                pat = ptail_pool.tile([128, GE], f32, tag="ptail")
                for e in range(GE):
                    b = g * GE + e
                    nc.tensor.transpose(
                        out=pat[:, e : e + 1],
                        in_=attn_g[0:1, e * L : (e + 1) * L],
                        identity=id1_sb[:],
                    )
                    nc.vector.tensor_copy(
                        out=Zt[b][:, e : e + 1], in_=pat[:, e : e + 1]
                    )
                pctx = ptail_pool.tile([GE, U], f32, tag="ptail")
                for e in range(GE):
                    b = g * GE + e
                    nc.tensor.matmul(
                        out=pctx[:],
                        lhsT=Zt[b][:, :],
                        rhs=va[b][:, :],
                        start=(e == 0),
                        stop=(e == GE - 1),
                    )
                ctx_sb = small.tile([GE, U], f32, tag="ctx")
                nc.vector.tensor_copy(out=ctx_sb[:], in_=pctx[:])
                nc.sync.dma_start(out=ctx_out[g * GE : (g + 1) * GE, :], in_=ctx_sb[:])

    nc.compile()
    return nc


def _get_program():
    global _PROGRAM
    if _PROGRAM is None:
        _PROGRAM = _build_program()
    return _PROGRAM


def make_in_maps(query, values, pos, W1, b1, W2, b2, V, bV):
    """Host-side shard + index prep. Pure numpy, no device work."""
    query = np.ascontiguousarray(np.asarray(query, np.float32))
    values = np.asarray(values, np.float32)
    pos = np.asarray(pos, np.int32)
    W1 = np.ascontiguousarray(np.asarray(W1, np.float32))
    W2 = np.ascontiguousarray(np.asarray(W2, np.float32))
    Vr = np.ascontiguousarray(np.asarray(V, np.float32).reshape(C, 128).T)
    b12 = (np.asarray(b1, np.float32) + np.asarray(b2, np.float32)).reshape(U)
    Zz = np.zeros((128, BP * GE), np.float32)
    offs = np.arange(L, dtype=np.int64)
    in_maps = []
    for core in range(NCORES):
        s = slice(core * BP, (core + 1) * BP)
        vshard = values[s].reshape(BP * T, U)
        valuesz = np.vstack([vshard, np.zeros((1, U), np.float32)])
        p = pos[s].astype(np.int64)
        start = np.maximum(p - HALF, 0)
        end = np.minimum(p + HALF, T)
        xlen = end - start
        # gidx[l, b]: row into valuesz; padded slots hit the zero row
        idx = start[None, :] + offs[:, None] + np.arange(BP)[None, :] * T
        idx = np.where(offs[:, None] < xlen[None, :], idx, BP * T)
        cs = np.zeros((128, 1172), np.float32)
        cs[:, 0:128] = np.eye(128, dtype=np.float32)
        cs[:, 128:132] = Vr
        for e in range(GE):
            cs[e, 140 + e * L : 140 + (e + 1) * L] = 1.0  # E4 block diag
        cs[32 : 32 + BP, 140:652] = query[s]
        cs[0, 656:1168] = b12
        cs[0, 1168:1172] = 1.0
        in_maps.append(
            {
                "valuesz": np.ascontiguousarray(valuesz),
                "W1": W1,
                "W2": W2,
                "consts": cs,
                "Zz": Zz,
                "gidx": np.ascontiguousarray(idx.astype(np.int32)),
            }
        )
    return in_maps


def run_on_hw(in_maps, trace=False, **kw):
    from concourse.bass_utils import run_bass_kernel_spmd

    nc = _get_program()
    return run_bass_kernel_spmd(nc, in_maps, list(range(NCORES)), trace=trace, **kw)


def kernel(query, values, pos, W1, b1, W2, b2, V, bV):
    in_maps = make_in_maps(query, values, pos, W1, b1, W2, b2, V, bV)
    res = run_on_hw(in_maps).results
    ctx = np.concatenate([r["ctx"] for r in res], axis=0)
    attn = np.concatenate([r["attn"] for r in res], axis=0)
    return ctx, attn.reshape(B, L, 1)
